# revision 2
# baseline (speedup 1.0000x reference)
"""GCN layer (X@W -> edge gather/scale -> segment-sum by dest -> +b -> relu)
as a Bass/Tile kernel on 8 Trainium2 NeuronCores.

v2 strategy (1D node partition, SPMD single program):
  - Source/table layout (sigma) = identity: core c computes XW for X rows
    [c*12500, (c+1)*12500) in bf16; two piece-AllGathers replicate the table
    into every core's DRAM as [2 pieces][8 cores][6272 rows] bf16, viewed as
    pairs [50176, 128] so 256B-aligned dma_gather works on bf16.
  - Destination layout (tau) = balanced assignment: nodes are packed into
    784 (core, block) bins of 128 so that per-(block, bucket) edge counts
    stay <= 512 (4 chunks of 128) -> ~3% stream padding instead of ~25%.
    bucket = (piece, parity) of the edge's SOURCE table position.
  - Edges sorted by (core, block, bucket, idx); per 128-edge chunk a
    selection matrix S[e,d] = val[e]*(dest[e]==d) is built in one bf16 DVE
    tensor_scalar (4x mode) and PE accumulates psum[128,64] += S^T @ G.
    G rows come from bf16 pair-gathers (elem 128 bf16 = 256B, parity
    selects columns 0:64 / 64:128).
  - Gathers round-robin over 4 SWDGE queues (3x descriptor throughput vs 1)
    with a depth throttle; piece-1 gathers trail piece-0 by 2 super-batches
    so they don't head-of-line block on the second AllGather.
"""

import math
from contextlib import ExitStack

import numpy as np
import ml_dtypes

import concourse.bacc as bacc
import concourse.mybir as mybir
import concourse.tile as tile
from concourse.bass import _add_dep_helper
from concourse.bass_utils import run_bass_kernel_spmd

BF16 = ml_dtypes.bfloat16

# Problem constants (hardcoded per contract; kernel.py must be self-contained).
N = 100000
E = 1600000
FIN = 256
FOUT = 64
NCORES = 8

P = 128
SHARD = N // NCORES           # 12500 source rows per core (sigma layout)
NBLK = 98                     # dest blocks per core (tau layout)
SHARD_PAD = NBLK * P          # 12544
PIECE_ROWS = SHARD_PAD // 2   # 6272 rows per core per AllGather piece
PIECE_PAIRS = PIECE_ROWS // 2  # 3136
WPAIRS = NCORES * PIECE_PAIRS  # 25088 pairs per piece window (< 32768)
TABLE_ROWS = NCORES * SHARD_PAD  # 100352
NBUCKET = 4                   # (piece, parity)
SB_BLOCKS = 6
NSB = NBLK // SB_BLOCKS + (1 if NBLK % SB_BLOCKS else 0)  # 17
KH = FIN // P
CHUNK_LIMIT = 4 * P           # target edges per (block, bucket)
NQUEUES = 4
GATHER_DEPTH = 8              # total gathers in flight (2 per queue)
PIECE_LAG = 1                 # piece-1 gathers trail piece-0 by this many sbs
EVICT_ACT = True              # bias-preload psum + Act relu eviction
S_POOL_EVERY = 0              # every k-th S-build on gpsimd (0 = all on DVE)
SPOOL_BUFS = 8                # S tile double-buffering depth
GATHER_SPLITS = 4             # gather instructions per (sb, bucket) segment


def _balance_bins(d):
    """Assign nodes (rows of d, [NNODES,4] bucket in-degree) to 784 bins of
    128 so that per-bin bucket sums stay <= CHUNK_LIMIT where possible.
    Returns member[784,128] node ids."""
    nbins = NCORES * NBLK
    nslots = nbins * P
    nn = d.shape[0]
    dd = np.zeros((nslots, 4), dtype=np.int64)
    dd[:nn] = d

    rng = np.random.default_rng(12345)
    order = rng.permutation(nslots)
    # LPT-snake rounds: heaviest nodes spread against lightest bins.
    tot = dd[order].sum(axis=1)
    order = order[np.argsort(-tot, kind="stable")]
    member = np.empty((nbins, P), dtype=np.int64)
    C = np.zeros((nbins, 4), dtype=np.int64)
    for r in range(P):
        batch = order[r * nbins:(r + 1) * nbins]
        bin_order = np.argsort(C.sum(axis=1), kind="stable")
        member[bin_order, r] = batch
        C[bin_order] += dd[batch]

    # Swap repair: greedily reduce total overflow sum(max(C - LIMIT, 0)).
    dmat = dd
    LIMIT = CHUNK_LIMIT
    NCAND = 48

    def ovf(v):
        return np.maximum(v - LIMIT, 0).sum(axis=-1)

    for _sweep in range(16):
        viol = np.argwhere(C > LIMIT)
        if len(viol) == 0:
            break
        progress = False
        for ob, b in viol:
            guard = 0
            while C[ob, b] > LIMIT and guard < 40:
                guard += 1
                mo = member[ob]
                dvals = dmat[mo, b]
                donor_is = np.argpartition(-dvals, 4)[:4]
                donor_is = donor_is[dvals[donor_is] > 0]
                if len(donor_is) == 0:
                    break
                slack = LIMIT - C[:, b]
                cands = np.argpartition(-slack, NCAND)[:NCAND]
                cands = cands[cands != ob]
                mem_c = member[cands]                      # [K,128]
                ris = np.argmin(dmat[mem_c[:, :], b], axis=1)
                backs = mem_c[np.arange(len(cands)), ris]  # [K]
                best = None
                for di in donor_is:
                    donor = mo[int(di)]
                    delta_vec = dmat[donor] - dmat[backs]  # [K,4]
                    newu = C[cands] + delta_vec
                    newo = C[ob][None, :] - delta_vec
                    gain = (ovf(newu) + ovf(newo)) - (ovf(C[cands]) + ovf(C[ob]))
                    bi = int(np.argmin(gain))
                    if gain[bi] < 0 and (best is None or gain[bi] < best[0]):
                        best = (gain[bi], int(di), donor, int(cands[bi]),
                                int(ris[bi]), int(backs[bi]),
                                newo[bi].copy(), newu[bi].copy())
                if best is None:
                    break
                _, di, donor, ub, ri, back, newo_v, newu_v = best
                member[ob, di] = back
                member[ub, ri] = donor
                C[ob] = newo_v
                C[ub] = newu_v
                progress = True
        if not progress:
            break
    return member, C


def _build_plan(edge_row, edge_col, edge_vals):
    """Host-side layout: balance dest bins, partition/sort/pad edges.
    Returns uniform structure + per-core staged arrays + output perm."""
    # ---- sigma (source/table) layout: identity ----
    c_src = edge_col // SHARD
    r_src = edge_col - c_src * SHARD
    piece = r_src // PIECE_ROWS          # 0/1
    parity = r_src & 1
    qloc = (r_src >> 1) - piece * PIECE_PAIRS
    idx16 = (c_src * PIECE_PAIRS + qloc).astype(np.int16)   # [0, 25088)
    bucket = piece * 2 + parity

    # ---- tau (dest) layout: balanced bins ----
    d = np.bincount(edge_row * 4 + bucket, minlength=N * 4).reshape(N, 4)
    member, C = _balance_bins(d)
    nbins = NCORES * NBLK

    # bins -> (core, blk): group similar chunk-need bins on the same blk
    need = np.ceil(C / P).astype(np.int64)          # [784, 4]
    rank = np.lexsort((need[:, 3], need[:, 2], need[:, 1], need[:, 0]))[::-1]
    core_of_bin = np.empty(nbins, dtype=np.int64)
    blk_of_bin = np.empty(nbins, dtype=np.int64)
    core_of_bin[rank] = np.arange(nbins) % NCORES
    blk_of_bin[rank] = np.arange(nbins) // NCORES

    # node -> (core, blk, pos)
    node_core = np.empty(nbins * P, dtype=np.int64)
    node_blk = np.empty(nbins * P, dtype=np.int64)
    node_pos = np.empty(nbins * P, dtype=np.int64)
    flat = member.reshape(-1)
    node_core[flat] = np.repeat(core_of_bin, P)
    node_blk[flat] = np.repeat(blk_of_bin, P)
    node_pos[flat] = np.tile(np.arange(P), nbins)
    # out_perm[core, blk*128+pos] = node id (>= N are ghosts)
    out_perm = np.empty((NCORES, SHARD_PAD), dtype=np.int64)
    out_perm[core_of_bin.repeat(P), (blk_of_bin.repeat(P)) * P +
             np.tile(np.arange(P), nbins)] = flat

    core_e = node_core[edge_row]
    blk_e = node_blk[edge_row]
    dest_e = node_pos[edge_row].astype(np.float32)

    # sort edges by (core, blk, bucket, idx)
    order = np.lexsort((idx16, bucket, blk_e, core_e))
    core_s = core_e[order]
    blk_s = blk_e[order]
    bucket_s = bucket[order]
    idx16_s = idx16[order]
    dest_s = dest_e[order]
    val_s = edge_vals[order].astype(np.float32)

    seg_key = (core_s * NBLK + blk_s) * NBUCKET + bucket_s
    counts = np.bincount(seg_key, minlength=NCORES * NBLK * NBUCKET).reshape(
        NCORES, NBLK, NBUCKET)
    chunks_bb = np.ceil(counts / P).astype(np.int64).max(axis=0)  # [NBLK,4]
    assert chunks_bb.sum(axis=1).min() >= 1
    cap_bb = chunks_bb * P

    # ---- static layout: stream order (sb, bucket, blk-in-sb, chunk) ----
    sb_of_blk = np.arange(NBLK) // SB_BLOCKS
    slot_off = np.zeros((NBLK, NBUCKET), dtype=np.int64)
    sb_b_len = np.zeros((NSB, NBUCKET), dtype=np.int64)
    for sb in range(NSB):
        blks = np.where(sb_of_blk == sb)[0]
        for b in range(NBUCKET):
            off = 0
            for bk in blks:
                slot_off[bk, b] = off
                off += cap_bb[bk, b]
            sb_b_len[sb, b] = off
    chunk_col0 = np.zeros((NSB, NBUCKET), dtype=np.int64)
    idx_col0 = np.zeros((NSB, NBUCKET), dtype=np.int64)
    ccur = icur = 0
    for sb in range(NSB):
        for b in range(NBUCKET):
            chunk_col0[sb, b] = ccur
            idx_col0[sb, b] = icur
            ccur += sb_b_len[sb, b] // P
            icur += sb_b_len[sb, b] // 16
    CTOT = ccur
    ITOT = icur

    first_of_seg = np.zeros(NCORES * NBLK * NBUCKET + 1, dtype=np.int64)
    np.cumsum(counts.reshape(-1), out=first_of_seg[1:])
    rank_e = np.arange(len(core_s)) - first_of_seg[seg_key]
    slot = (chunk_col0[sb_of_blk[blk_s], bucket_s] * P
            + slot_off[blk_s, bucket_s] + rank_e)

    idx_streams = np.zeros((NCORES, CTOT * P), dtype=np.int16)
    dest_streams = np.zeros((NCORES, CTOT * P), dtype=np.float32)
    val_streams = np.zeros((NCORES, CTOT * P), dtype=np.float32)
    for c in range(NCORES):
        m = core_s == c
        idx_streams[c, slot[m]] = idx16_s[m]
        dest_streams[c, slot[m]] = dest_s[m]
        val_streams[c, slot[m]] = val_s[m]

    dest_np = dest_streams.reshape(NCORES, CTOT, P).transpose(0, 2, 1)
    val_np = val_streams.reshape(NCORES, CTOT, P).transpose(0, 2, 1)
    dest_np = np.ascontiguousarray(dest_np)
    val_np = np.ascontiguousarray(val_np)

    idx_np = np.zeros((NCORES, P, ITOT), dtype=np.int16)
    for sb in range(NSB):
        for b in range(NBUCKET):
            L = int(sb_b_len[sb, b])
            if L == 0:
                continue
            s0 = int(chunk_col0[sb, b]) * P
            i0 = int(idx_col0[sb, b])
            seg = idx_streams[:, s0:s0 + L].reshape(NCORES, L // 16, 16)
            seg = seg.transpose(0, 2, 1)
            idx_np[:, :, i0:i0 + L // 16] = np.tile(seg, (1, 8, 1))

    # per-block chunk list in (bucket, chunk) order:
    # (bucket, j_local_in_gather_tile, global_chunk_col)
    blk_chunks = []
    for bk in range(NBLK):
        sb = int(sb_of_blk[bk])
        lst = []
        for b in range(NBUCKET):
            nch = int(chunks_bb[bk, b])
            j0 = int(slot_off[bk, b]) // P
            c0 = int(chunk_col0[sb, b]) + j0
            for k in range(nch):
                lst.append((b, j0 + k, c0 + k))
        blk_chunks.append(lst)

    struct = dict(
        chunks_bb=chunks_bb, sb_b_len=sb_b_len, chunk_col0=chunk_col0,
        idx_col0=idx_col0, CTOT=CTOT, ITOT=ITOT, blk_chunks=blk_chunks,
        sb_of_blk=sb_of_blk,
    )
    return struct, idx_np, dest_np, val_np, out_perm


_NO_SPLIT = ("InstEventSemaphore", "InstDrain", "InstCollectiveCompute",
             "InstCall", "InstUnconditionalBranch", "InstConditionalBranch")


def _split_excess_waits(nc):
    """TRN2 instructions tolerate very few sync waits; move all but one
    semaphore wait onto wait-only InstEventSemaphore instructions inserted
    just before on the same engine."""
    for blk in nc.main_func.blocks:
        out = []
        for ins in blk.instructions:
            si = ins.sync_info
            tn = type(ins).__name__
            if si is None or tn in _NO_SPLIT or len(si.on_wait) <= 1:
                out.append(ins)
                continue
            waits = list(si.on_wait)
            keep, excess = waits[:1], waits[1:]
            while excess:
                batch, excess = excess[:2], excess[2:]
                ev = mybir.InstEventSemaphore(
                    name=nc.get_next_instruction_name(), ins=[], outs=[])
                ev.engine = ins.engine
                ev.sync_info = mybir.SyncInfo(on_wait=batch, on_update=[])
                out.append(ev)
            ins.sync_info = mybir.SyncInfo(
                on_wait=keep, on_update=list(si.on_update))
            out.append(ins)
        blk.instructions[:] = out


def _build_nc(struct, variant="full"):
    st = struct
    CTOT, ITOT = st["CTOT"], st["ITOT"]
    nc = bacc.Bacc("TRN2", target_bir_lowering=False, debug=False,
                   num_devices=NCORES, num_swdge_queues=NQUEUES)
    f32 = mybir.dt.float32
    bf16 = mybir.dt.bfloat16
    i16 = mybir.dt.int16

    xt_sh = nc.dram_tensor("xt_sh", [FIN, SHARD_PAD], bf16, kind="ExternalInput")
    w_in = nc.dram_tensor("w_in", [FIN, FOUT], bf16, kind="ExternalInput")
    b_rep = nc.dram_tensor("b_rep", [P, FOUT], f32, kind="ExternalInput")
    iota_in = nc.dram_tensor("iota_in", [P, P], bf16, kind="ExternalInput")
    idx_in = nc.dram_tensor("idx_in", [P, ITOT], i16, kind="ExternalInput")
    dest_in = nc.dram_tensor("dest_in", [P, CTOT], f32, kind="ExternalInput")
    val_in = nc.dram_tensor("val_in", [P, CTOT], f32, kind="ExternalInput")

    xw_sh = nc.dram_tensor("xw_sh", [SHARD_PAD, FOUT], bf16, kind="Internal")
    table = nc.dram_tensor("table", [TABLE_ROWS, FOUT], bf16, kind="Internal",
                           addr_space="Shared")
    out_sh = nc.dram_tensor("out_sh", [SHARD_PAD, FOUT], f32,
                            kind="ExternalOutput")

    with tile.TileContext(nc) as tc, ExitStack() as ctx:
        consts = ctx.enter_context(tc.tile_pool(name="consts", bufs=1))
        gpool = ctx.enter_context(tc.tile_pool(name="gpool", bufs=3))
        spool = ctx.enter_context(tc.tile_pool(name="spool", bufs=SPOOL_BUFS))
        opool = ctx.enter_context(tc.tile_pool(name="opool", bufs=4))
        xpool = ctx.enter_context(tc.tile_pool(name="xpool", bufs=2))
        pmpool = ctx.enter_context(
            tc.tile_pool(name="pmpool", bufs=2, space="PSUM"))
        popool = ctx.enter_context(
            tc.tile_pool(name="popool", bufs=6, space="PSUM"))

        iota_t = consts.tile([P, P], bf16)
        nc.sync.dma_start(out=iota_t[:], in_=iota_in[:])
        brep_t = consts.tile([P, FOUT], f32)
        nc.sync.dma_start(out=brep_t[:], in_=b_rep[:])
        w_t = []
        for h in range(KH):
            wt = consts.tile([P, FOUT], bf16, tag=f"w{h}")
            nc.sync.dma_start(out=wt[:], in_=w_in[h * P:(h + 1) * P, :])
            w_t.append(wt)
        dst_all = consts.tile([P, CTOT], f32, tag="dstall")
        nc.sync.dma_start(out=dst_all[:], in_=dest_in[:])
        vl_all = consts.tile([P, CTOT], f32, tag="vlall")
        nc.sync.dma_start(out=vl_all[:], in_=val_in[:])
        idx_all = consts.tile([P, ITOT], i16, tag="idxall")
        nc.sync.dma_start(out=idx_all[:], in_=idx_in[:])

        # ---------------- phase 1: GEMM shard (bf16) ----------------
        GRP = 1792
        assert SHARD_PAD % GRP == 0 and GRP % P == 0
        for g in range(SHARD_PAD // GRP):
            xts = []
            for h in range(KH):
                xt = xpool.tile([P, GRP], bf16, tag=f"xt{h}")
                nc.sync.dma_start(
                    out=xt[:], in_=xt_sh[h * P:(h + 1) * P,
                                         g * GRP:(g + 1) * GRP])
                xts.append(xt)
            for c in range(GRP // P):
                bk = g * (GRP // P) + c
                mm = pmpool.tile([P, FOUT], f32, tag="mm")
                for h in range(KH):
                    nc.tensor.matmul(
                        out=mm[:], lhsT=xts[h][:, c * P:(c + 1) * P],
                        rhs=w_t[h][:], start=(h == 0), stop=(h == KH - 1))
                om = opool.tile([P, FOUT], bf16, tag="om")
                nc.vector.tensor_copy(out=om[:], in_=mm[:])
                nc.sync.dma_start(
                    out=xw_sh[bk * P:(bk + 1) * P, :], in_=om[:])

        # ---------------- phase 2: piece AllGathers ----------------
        if variant == "p1":
            ob = opool.tile([P, FOUT], f32, tag="ob")
            nc.vector.tensor_copy(out=ob[:], in_=brep_t[:])
            nc.sync.dma_start(out=out_sh[:P, :], in_=ob[:])
        for p in (() if variant == "p1" else range(2)):
            nc.gpsimd.collective_compute(
                kind="AllGather", op=mybir.AluOpType.bypass,
                replica_groups=[list(range(NCORES))],
                ins=[xw_sh[p * PIECE_ROWS:(p + 1) * PIECE_ROWS, :]],
                outs=[table[p * (TABLE_ROWS // 2):(p + 1) * (TABLE_ROWS // 2), :]],
            )

        # ---------------- phase 3: gather + segment-sum ----------------
        sb_b_len = st["sb_b_len"]
        chunk_col0 = st["chunk_col0"]
        idx_col0 = st["idx_col0"]
        blk_chunks = st["blk_chunks"]
        sb_of_blk = st["sb_of_blk"]

        if variant in ("p12",):
            ob = opool.tile([P, FOUT], f32, tag="ob")
            nc.vector.tensor_copy(out=ob[:], in_=brep_t[:])
            nc.sync.dma_start(out=out_sh[:P, :], in_=ob[:])

        gather_insts = []
        gq = [0]
        seng = [0]

        def issue_gathers(sb, buckets, gts_by_sb):
            gts = gts_by_sb.setdefault(sb, [None] * NBUCKET)
            for b in buckets:
                L = int(sb_b_len[sb, b])
                if L == 0:
                    continue
                nch = L // P
                icol = int(idx_col0[sb, b])
                piece = b // 2
                gt = gpool.tile([P, nch * P], bf16, tag=f"g{b}")
                in_pairs = table[piece * (TABLE_ROWS // 2):
                                 (piece + 1) * (TABLE_ROWS // 2), :].rearrange(
                    "(q t) f -> q (t f)", t=2)
                nsp = min(GATHER_SPLITS, nch)
                bounds = [round(i * nch / nsp) for i in range(nsp + 1)]
                for c0, c1 in zip(bounds[:-1], bounds[1:]):
                    if c1 == c0:
                        continue
                    gi = nc.gpsimd.dma_gather(
                        out_ap=gt[:, c0 * P:c1 * P].rearrange(
                            "p (c f) -> p c f", f=P),
                        in_ap=in_pairs,
                        idxs_ap=idx_all[:, icol + c0 * 8:icol + c1 * 8],
                        num_idxs=(c1 - c0) * P,
                        num_idxs_reg=(c1 - c0) * P,
                        elem_size=P,
                        single_packet=False,
                        queue_num=gq[0] % NQUEUES,
                    )
                    gq[0] += 1
                    if len(gather_insts) >= GATHER_DEPTH:
                        _add_dep_helper(gi.ins, gather_insts[-GATHER_DEPTH],
                                        sync=True, reason="swdge throttle")
                    gather_insts.append(gi.ins)
                gts[b] = gt

        def process_sb(sb, gts_by_sb):
            gts = gts_by_sb.pop(sb)
            blks = [bk for bk in range(NBLK) if sb_of_blk[bk] == sb]
            po_of = {}
            n_of = {}
            k_of = {}
            for bk in blks:
                po_of[bk] = popool.tile([P, FOUT], f32, tag="po",
                                        name=f"po_sb{sb}_b{bk}")
                n_of[bk] = len(blk_chunks[bk])
                k_of[bk] = 0
                if EVICT_ACT:
                    nc.scalar.copy(out=po_of[bk][:], in_=brep_t[:])
            # bucket-major so piece-1 chunks come last
            for b in range(NBUCKET):
                parity = b & 1
                for bi, bk in enumerate(blks):
                    for (bb, j, gcol) in blk_chunks[bk]:
                        if bb != b:
                            continue
                        s_t = spool.tile([P, P], bf16, tag="s")
                        seng[0] += 1
                        eng = (nc.gpsimd if S_POOL_EVERY and
                               seng[0] % S_POOL_EVERY == 0 else nc.vector)
                        eng.tensor_scalar(
                            out=s_t[:], in0=iota_t[:],
                            scalar1=dst_all[:, gcol:gcol + 1],
                            scalar2=vl_all[:, gcol:gcol + 1],
                            op0=mybir.AluOpType.is_equal,
                            op1=mybir.AluOpType.mult,
                        )
                        k = k_of[bk]
                        nc.tensor.matmul(
                            out=po_of[bk][:],
                            lhsT=s_t[:],
                            rhs=gts[b][:, j * P + parity * FOUT:
                                       j * P + parity * FOUT + FOUT],
                            start=(k == 0 and not EVICT_ACT),
                            stop=(k == n_of[bk] - 1),
                        )
                        k_of[bk] = k + 1
            for bi, bk in enumerate(blks):
                ob = opool.tile([P, FOUT], f32, tag="ob")
                if EVICT_ACT:
                    nc.scalar.activation(
                        out=ob[:], in_=po_of[bk][:],
                        func=mybir.ActivationFunctionType.Relu)
                else:
                    nc.vector.tensor_tensor(
                        out=ob[:], in0=po_of[bk][:],
                        in1=brep_t[:], op=mybir.AluOpType.add)
                    nc.vector.tensor_scalar(
                        out=ob[:], in0=ob[:], scalar1=0.0, scalar2=None,
                        op0=mybir.AluOpType.max)
                nc.sync.dma_start(
                    out=out_sh[bk * P:(bk + 1) * P, :], in_=ob[:])

        if variant == "dbg":
            # dump sample table blocks, gather tiles, and one S matrix into
            # out_sh rows for host comparison
            row = [0]

            def dump(tile_ap, cast=True):
                ob = opool.tile([P, FOUT], f32, tag="ob")
                nc.vector.tensor_copy(out=ob[:], in_=tile_ap)
                nc.sync.dma_start(
                    out=out_sh[row[0] * P:(row[0] + 1) * P, :], in_=ob[:])
                row[0] += 1

            tpool = ctx.enter_context(tc.tile_pool(name="tpool", bufs=2))
            for r0 in (0, 6272, 12544, 50176, 50176 + 6272):
                tt = tpool.tile([P, FOUT], bf16, tag="tt")
                nc.sync.dma_start(out=tt[:], in_=table[r0:r0 + P, :])
                dump(tt[:])
            gts_by_sb = {}
            issue_gathers(0, (0, 1, 2, 3), gts_by_sb)
            for b in range(NBUCKET):
                gt = gts_by_sb[0][b]
                dump(gt[:, :FOUT])          # chunk 0, parity 0 slice
                dump(gt[:, FOUT:2 * FOUT])  # chunk 0, parity 1 slice
            # one S matrix from the first chunk of (sb0, b0)
            gcol0 = int(chunk_col0[0, 0])
            s_t = spool.tile([P, P], bf16, tag="s")
            nc.vector.tensor_scalar(
                out=s_t[:], in0=iota_t[:],
                scalar1=dst_all[:, gcol0:gcol0 + 1],
                scalar2=vl_all[:, gcol0:gcol0 + 1],
                op0=mybir.AluOpType.is_equal,
                op1=mybir.AluOpType.mult,
            )
            dump(s_t[:, :FOUT])
            dump(s_t[:, FOUT:2 * FOUT])
            # one chunk matmul into a fresh psum bank, two column slices
            po = popool.tile([P, 2 * FOUT], f32, tag="po", name="po_dbg")
            nc.tensor.matmul(out=po[:, :FOUT], lhsT=s_t[:],
                             rhs=gts_by_sb[0][0][:, :FOUT],
                             start=True, stop=False)
            nc.tensor.matmul(out=po[:, FOUT:2 * FOUT], lhsT=s_t[:],
                             rhs=gts_by_sb[0][0][:, :FOUT],
                             start=True, stop=False)
            nc.tensor.matmul(out=po[:, :FOUT], lhsT=s_t[:],
                             rhs=gts_by_sb[0][0][:, FOUT:2 * FOUT],
                             start=False, stop=True)
            nc.tensor.matmul(out=po[:, FOUT:2 * FOUT], lhsT=s_t[:],
                             rhs=gts_by_sb[0][0][:, FOUT:2 * FOUT],
                             start=False, stop=True)
            dump(po[:, :FOUT])
            dump(po[:, FOUT:2 * FOUT])

        if variant not in ("p12", "p1", "dbg"):
            gts_by_sb = {}
            emitted = []
            for sb in range(NSB):
                issue_gathers(sb, (0, 1), gts_by_sb)
                if sb >= PIECE_LAG:
                    issue_gathers(sb - PIECE_LAG, (2, 3), gts_by_sb)
                    emitted.append(sb - PIECE_LAG)
                    if variant != "p12g":
                        process_sb(sb - PIECE_LAG, gts_by_sb)
            for sb in range(max(NSB - PIECE_LAG, 0), NSB):
                issue_gathers(sb, (2, 3), gts_by_sb)
                emitted.append(sb)
                if variant != "p12g":
                    process_sb(sb, gts_by_sb)
            if variant == "p12g":
                ob = opool.tile([P, FOUT], f32, tag="ob")
                nc.vector.tensor_copy(
                    out=ob[:], in_=gts_by_sb[NSB - 1][0][:, :FOUT])
                nc.sync.dma_start(out=out_sh[:P, :], in_=ob[:])

    nc.compile()
    _split_excess_waits(nc)
    return nc


def _prepare(X, edge_row, edge_col, edge_vals, W, b, variant="full"):
    X = np.asarray(X, dtype=np.float32)
    edge_row = np.asarray(edge_row, dtype=np.int64)
    edge_col = np.asarray(edge_col, dtype=np.int64)
    edge_vals = np.asarray(edge_vals, dtype=np.float32)
    W = np.asarray(W, dtype=np.float32)
    b = np.asarray(b, dtype=np.float32)

    struct, idx_np, dest_np, val_np, out_perm = _build_plan(
        edge_row, edge_col, edge_vals)
    nc = _build_nc(struct, variant=variant)

    b_rep = np.tile(b[None, :], (P, 1)).astype(np.float32)
    iota = np.tile(np.arange(P, dtype=np.float32)[None, :], (P, 1)).astype(BF16)
    W16 = W.astype(BF16)

    in_maps = []
    for c in range(NCORES):
        xt_pad = np.zeros((FIN, SHARD_PAD), dtype=BF16)
        xt_pad[:, :SHARD] = X[c * SHARD:(c + 1) * SHARD].T.astype(BF16)
        in_maps.append({
            "xt_sh": xt_pad, "w_in": W16, "b_rep": b_rep,
            "iota_in": iota, "idx_in": idx_np[c], "dest_in": dest_np[c],
            "val_in": val_np[c],
        })
    return nc, in_maps, out_perm


def _assemble(results, out_perm):
    out = np.empty((N, FOUT), dtype=np.float32)
    for c in range(NCORES):
        ids = out_perm[c]
        m = ids < N
        out[ids[m]] = results[c]["out_sh"][m]
    return out


def kernel(X, edge_row, edge_col, edge_vals, W, b):
    nc, in_maps, out_perm = _prepare(X, edge_row, edge_col, edge_vals, W, b)
    res = run_bass_kernel_spmd(nc, in_maps, core_ids=list(range(NCORES)))
    return _assemble(res.results, out_perm)
